# revision 1
# baseline (speedup 1.0000x reference)
"""DigitCaps kernel for 8 Trainium2 NeuronCores.

Math (per batch b):
    U_hat[b,d,n,j] = sum_i W[d,n,j,i] * u[b,n,i]
    A_sum[b,d,m]   = sum_n U_hat[b,d,n,:] . U_hat[b,d,m,:] / sqrt(dp)
                   = s[b,d,:] . U_hat[b,d,m,:] / sqrt(dp),  s = sum_n U_hat
    C              = softmax_d(A_sum)
    S[b,d,j]       = sum_m (B_prior[d,m] + C[b,d,m]) * U_hat[b,d,m,j]
    out            = squash(S)

The huge [B,D,N,N] similarity matrix collapses because it is immediately
summed over n - only the n-sum s of U_hat is needed.

Sharding: data-parallel over batch, 2 batches per core, W/B_prior replicated.
Inputs are pre-arranged on the host into per-tile layouts so every device DMA
reads fully contiguous memory.

Per-core layout: n-tiles of 128 on partitions.
    W_sb[nt]  : [n=128, (d,j,i)=1280]   (host-arranged, contiguous 5KB rows)
    U2[nt]    : [n=128, (b,d,j)=320]    multiply-accumulate chain over i
                (DVE TensorScalarPtr; 4 half-chains on GpSimd as mult+tree)
    s         : ones.T @ U2 fp32        (PE partition-reduce, per-batch-half
                                         PSUM groups; all rows equal s)
    then per n-tile (pipelined): A_sum (GpSimd mult + DVE reduce) -> exp
    (ACT, scale=1/sqrt(dp), table preloaded) -> softmax_d norm (DVE) ->
    +B_prior -> S matmul (PE, bf16, PSUM accum);
    diagonal extract via iota mask; squash with Newton sqrt on DVE (keeps
    the Exp ACT table resident - no table reloads in the tail).
"""

import math
import numpy as np

import concourse.bacc as bacc
import concourse.bass as bass
import concourse.tile as tile
from concourse import mybir
from concourse.bass_utils import run_bass_kernel_spmd

F32 = mybir.dt.float32
I32 = mybir.dt.int32
AX = mybir.AxisListType
OP = mybir.AluOpType
ACTF = mybir.ActivationFunctionType

B, N, DP = 16, 1152, 8
D, DD = 10, 16
NCORES = 8
BPC = B // NCORES            # 2 batches per core
NT = N // 128                # 9 n-tiles
FW = D * DD * DP             # 1280 W free size
FD = D * DD                  # 160 per-batch U2 free size
FU = BPC * FD                # 320 U2 free size
NBD = BPC * D                # 20 (b,d) pairs
EPS = 1e-7
INV_SQRT_DP = 1.0 / math.sqrt(DP)


def _build_kernel(tc: "tile.TileContext", out_ap, WUB):
    nc = tc.nc
    with (
        tc.tile_pool(name="wpool", bufs=NT) as wpool,
        tc.tile_pool(name="tapool", bufs=6) as tapool,
        tc.tile_pool(name="ppool", bufs=4) as ppool,
        tc.tile_pool(name="smpool", bufs=2) as smpool,
        tc.tile_pool(name="persist", bufs=1) as persist,
        tc.tile_pool(name="psum_s", bufs=1, space="PSUM") as psum_s,
        tc.tile_pool(name="psum_S2", bufs=1, space="PSUM") as psum_S2,
    ):
        BF16 = mybir.dt.bfloat16
        ones_t = persist.tile([128, 128], F32, tag="ones")
        nc.vector.memset(ones_t[:], 1.0)

        u2_all = persist.tile([128, NT * FU], F32, tag="u2all")
        u2bf_all = persist.tile([128, NT * FU], BF16, tag="u2bfall")
        cbbf_all = persist.tile([128, NT * NBD], BF16, tag="cbbfall")
        cb_all = persist.tile([128, NT * NBD], F32, tag="cball")
        e_all = persist.tile([128, NT * NBD], F32, tag="eall")
        z_all = persist.tile([128, NT * BPC], F32, tag="zall")
        zr_all = persist.tile([128, NT * BPC], F32, tag="zrall")

        s_ps_0 = psum_s.tile([128, FD], F32, tag="sps0")
        s_ps_1 = psum_s.tile([128, FD], F32, tag="sps1")
        s_ps_b = [s_ps_0, s_ps_1]

        # preload the Exp ACT table while ACT is idle (hides the ~1.3us
        # table load that would otherwise land in the phase-2 critical path)
        warm_t = persist.tile([1, 1], F32, tag="warm")
        nc.vector.memset(warm_t[:], 0.0)
        nc.scalar.activation(warm_t[:], warm_t[:], ACTF.Exp)

        # ---- phase 1: load; U2 votes via i-chain; running s on PE ----
        ACT_TILES = ()  # these tiles route products via ACT + GpSimd
        # (nt, b) half-chains routed to GpSimd (mult + tree, all Pool-legal)
        POOL_HALVES = {(1, 1), (3, 1), (5, 1), (7, 1)}
        w_tiles = []
        for nt in range(NT):
            w_t = wpool.tile([128, FW + BPC * DP + D], F32, tag="w")
            w_tiles.append(w_t)
            nc.sync.dma_start(w_t[:], WUB[nt])
            u_t = w_t[:, FW: FW + BPC * DP]

            # U2[n,(b,d,j)] += W[n,(d,j,i)] * u[n,(b,i)]  accumulated over i.
            # TensorScalarPtr is DVE-only on trn2 (walrus rejects it on Pool),
            # so offload tiles via ACT products + GpSimd tree-reduce instead.
            w_3 = w_t[:, :FW].rearrange("p (dj i) -> p dj i", dj=FD, i=DP)
            if nt in ACT_TILES:
                # products P[n,(b,dj,i)] on ACT (Copy with per-partition
                # scale), then i-tree-reduce on GpSimd
                pp = ppool.tile([128, BPC * FW], F32, tag="pp")
                pp_v = pp[:].rearrange(
                    "p (b dj i) -> p b dj i", b=BPC, dj=FD, i=DP
                )
                for b in range(BPC):
                    for i in range(DP):
                        nc.scalar.activation(
                            pp_v[:, b, :, i],
                            w_3[:, :, i],
                            ACTF.Copy,
                            scale=u_t[:, b * DP + i: b * DP + i + 1],
                        )
                t1 = ppool.tile([128, BPC * FD * 4], F32, tag="t1")
                t1_v = t1[:].rearrange("p (g i) -> p g i", g=BPC * FD, i=4)
                pp_g = pp[:].rearrange("p (g i) -> p g i", g=BPC * FD, i=DP)
                nc.gpsimd.tensor_tensor(
                    t1_v, pp_g[:, :, 0:4], pp_g[:, :, 4:8], OP.add
                )
                t2 = ppool.tile([128, BPC * FD * 2], F32, tag="t2")
                t2_v = t2[:].rearrange("p (g i) -> p g i", g=BPC * FD, i=2)
                nc.gpsimd.tensor_tensor(
                    t2_v, t1_v[:, :, 0:2], t1_v[:, :, 2:4], OP.add
                )
                nc.gpsimd.tensor_tensor(
                    u2_all[:, nt * FU:(nt + 1) * FU].rearrange(
                        "p (g i) -> p g i", g=BPC * FD, i=1
                    ),
                    t2_v[:, :, 0:1],
                    t2_v[:, :, 1:2],
                    OP.add,
                )
            else:
                for b in range(BPC):
                    u2_sl = u2_all[:, nt * FU + b * FD: nt * FU + (b + 1) * FD]
                    if (nt, b) in POOL_HALVES:
                        # GpSimd route: one big mult + 3 tree-adds over i
                        pp = ppool.tile([128, FW], F32, tag="pp")
                        pp_v = pp[:].rearrange("p (g i) -> p g i", g=FD, i=DP)
                        u_bc = (
                            u_t[:, b * DP:(b + 1) * DP]
                            .unsqueeze(1)
                            .broadcast_to([128, FD, DP])
                        )
                        nc.gpsimd.tensor_tensor(pp_v, w_3, u_bc, OP.mult)
                        t1 = ppool.tile([128, FD * 4], F32, tag="t1")
                        t1_v = t1[:].rearrange("p (g i) -> p g i", g=FD, i=4)
                        nc.gpsimd.tensor_tensor(
                            t1_v, pp_v[:, :, 0:4], pp_v[:, :, 4:8], OP.add
                        )
                        t2 = ppool.tile([128, FD * 2], F32, tag="t2")
                        t2_v = t2[:].rearrange("p (g i) -> p g i", g=FD, i=2)
                        nc.gpsimd.tensor_tensor(
                            t2_v, t1_v[:, :, 0:2], t1_v[:, :, 2:4], OP.add
                        )
                        nc.gpsimd.tensor_tensor(
                            u2_sl.rearrange("p (g i) -> p g i", g=FD, i=1),
                            t2_v[:, :, 0:1],
                            t2_v[:, :, 1:2],
                            OP.add,
                        )
                        nc.tensor.matmul(
                            s_ps_b[b][:],
                            ones_t[:],
                            u2_sl,
                            start=(nt == 0),
                            stop=(nt == NT - 1),
                        )
                        continue
                    # first product on ACT (Copy with per-partition scale)
                    # frees two DVE ops per tile
                    nc.scalar.activation(
                        u2_sl,
                        w_3[:, :, 0],
                        ACTF.Copy,
                        scale=u_t[:, b * DP: b * DP + 1],
                    )
                    for i in range(1, DP):
                        nc.vector.scalar_tensor_tensor(
                            u2_sl,
                            w_3[:, :, i],
                            u_t[:, b * DP + i: b * DP + i + 1],
                            u2_sl,
                            OP.mult,
                            OP.add,
                        )
                    # s accumulation for this half-chain (fp32, PE idle;
                    # column-split groups give finer start dependencies)
                    nc.tensor.matmul(
                        s_ps_b[b][:],
                        ones_t[:],
                        u2_sl,
                        start=(nt == 0),
                        stop=(nt == NT - 1),
                    )



        # ---- phase 2 (pipelined per n-tile): A_sum -> softmax_d -> +B_prior
        #      -> S matmul ----
        # s copy to SBUF so GpSimd (no PSUM access) can read it (DVE: the
        # chain engine is free here and ACT's queue is backlogged)
        s_sb = persist.tile([128, FU], F32, tag="ssb")
        for b in range(BPC):
            nc.vector.tensor_copy(s_sb[:, b * FD:(b + 1) * FD], s_ps_b[b][:])

        # bf16 shadow of U2 for the S2 matmuls - cast lazily here, where ACT
        # is otherwise idle and off the phase-1 -> phase-2 critical path
        for nt in range(NT):
            nc.scalar.copy(
                u2bf_all[:, nt * FU:(nt + 1) * FU],
                u2_all[:, nt * FU:(nt + 1) * FU],
            )

        S2_ps = psum_S2.tile([NBD, FU], F32, tag="S2")
        POOL_TILES = (2, 3, 4, 5, 6, 7, 8)  # TA on GpSimd for these n-tiles
        for nt in range(NT):
            u2_sl = u2_all[:, nt * FU:(nt + 1) * FU]
            a_sl = e_all[:, nt * NBD:(nt + 1) * NBD]  # staging (overwritten by exp)
            ta = tapool.tile([128, FU], F32, tag="ta")
            if nt in POOL_TILES:
                nc.gpsimd.tensor_tensor(ta[:], u2_sl, s_sb[:], OP.mult)
            else:
                nc.vector.tensor_tensor(ta[:], u2_sl, s_sb[:], OP.mult)
            nc.vector.tensor_reduce(
                a_sl,
                ta[:].rearrange("p (g j) -> p g j", g=NBD, j=DD),
                AX.X,
                OP.add,
            )
            # E = exp(A / sqrt(dp))
            nc.scalar.activation(a_sl, a_sl, ACTF.Exp, scale=INV_SQRT_DP)
            # z[(b)] = sum_d E ; zr = 1/z
            z_sl = z_all[:, nt * BPC:(nt + 1) * BPC]
            zr_sl = zr_all[:, nt * BPC:(nt + 1) * BPC]
            nc.vector.tensor_reduce(
                z_sl,
                a_sl.rearrange("p (b d) -> p b d", b=BPC, d=D),
                AX.X,
                OP.add,
            )
            nc.vector.reciprocal(zr_sl, z_sl)
            # cb = E * zr + B_prior, written directly as bf16 for the matmul
            cbbf_sl = cbbf_all[:, nt * NBD:(nt + 1) * NBD]
            for b in range(BPC):
                nc.vector.scalar_tensor_tensor(
                    cbbf_sl[:, b * D:(b + 1) * D],
                    a_sl[:, b * D:(b + 1) * D],
                    zr_sl[:, b: b + 1],
                    w_tiles[nt][:, FW + BPC * DP: FW + BPC * DP + D],
                    OP.mult,
                    OP.add,
                )
            # S2 += cb.T @ U2 (bf16 operands, fp32 PSUM accumulate)
            nc.tensor.matmul(
                S2_ps[:],
                cbbf_sl,
                u2bf_all[:, nt * FU:(nt + 1) * FU],
                start=(nt == 0),
                stop=(nt == NT - 1),
            )

        # ---- phase 3: extract diagonal (b,d)=(b',d') via iota mask ----
        iota_t = persist.tile([NBD, FU], I32, tag="iota")
        nc.gpsimd.iota(
            iota_t[:], pattern=[[1, NBD], [0, DD]], base=0, channel_multiplier=-1
        )
        mask_t = persist.tile([NBD, FU], F32, tag="mask")
        nc.vector.tensor_scalar(mask_t[:], iota_t[:], 0, None, OP.is_equal)

        sm_t = smpool.tile([NBD, FU], F32, tag="sm")
        nc.vector.tensor_tensor(sm_t[:], S2_ps[:], mask_t[:], OP.mult)
        s_diag = persist.tile([NBD, DD], F32, tag="sdiag")
        nc.vector.tensor_reduce(
            s_diag[:],
            sm_t[:].rearrange("p (g j) -> p j g", g=NBD, j=DD),
            AX.X,
            OP.add,
        )

        # ---- phase 4: squash ----
        ss_t = persist.tile([NBD, DD], F32, tag="ss")
        nrm2 = persist.tile([NBD, 1], F32, tag="nrm2")
        nc.vector.tensor_tensor(ss_t[:], s_diag[:], s_diag[:], OP.mult)
        nc.vector.tensor_reduce(nrm2[:], ss_t[:], AX.X, OP.add)
        # norm via DVE Newton sqrt (bit-hack seed + 2 iterations) - keeps the
        # Exp ACT table resident (no sqrt/exp table reload in the tail)
        # norm via one Halley iteration from the bit-hack seed (cubic:
        # 3.5e-2 seed error -> ~4e-5), all on DVE
        nrm = persist.tile([NBD, 1], F32, tag="nrm")
        seed_i = persist.tile([NBD, 1], I32, tag="seedi")
        nc.vector.tensor_scalar(
            seed_i[:], nrm2[:].bitcast(I32), 1, None, OP.logical_shift_right
        )
        nc.vector.tensor_scalar(seed_i[:], seed_i[:], 0x1FBD1DF5, None, OP.add)
        seed_f = seed_i[:].bitcast(F32)
        y2 = persist.tile([NBD, 1], F32, tag="y2")
        nc.vector.tensor_tensor(y2[:], seed_f, seed_f, OP.mult)
        hnum = persist.tile([NBD, 1], F32, tag="hnum")
        nc.vector.scalar_tensor_tensor(hnum[:], nrm2[:], 3.0, y2[:], OP.mult, OP.add)
        hden = persist.tile([NBD, 1], F32, tag="hden")
        nc.vector.scalar_tensor_tensor(hden[:], y2[:], 3.0, nrm2[:], OP.mult, OP.add)
        nwr = persist.tile([NBD, 1], F32, tag="nwr")
        nc.vector.reciprocal(nwr[:], hden[:])
        nwt = persist.tile([NBD, 1], F32, tag="nwt")
        nc.vector.tensor_tensor(nwt[:], hnum[:], nwr[:], OP.mult)
        nc.vector.tensor_tensor(nrm[:], seed_f, nwt[:], OP.mult)
        # coef = 1 - 1/(e^r + eps) ~= 1 - e^-r  (abs diff <= eps*e^-2r <= 1e-7)
        en = persist.tile([NBD, 1], F32, tag="en")
        nc.scalar.activation(en[:], nrm[:], ACTF.Exp, scale=-1.0)
        coef = persist.tile([NBD, 1], F32, tag="coef")
        nc.vector.tensor_scalar(coef[:], en[:], -1.0, 1.0, OP.mult, OP.add)
        # norm >= O(1) here: the reference's +1e-7 on the divisor is below
        # fp32 resolution of the result - divide by nrm directly
        r2 = persist.tile([NBD, 1], F32, tag="r2")
        nc.vector.reciprocal(r2[:], nrm[:])
        fac = persist.tile([NBD, 1], F32, tag="fac")
        nc.vector.tensor_tensor(fac[:], coef[:], r2[:], OP.mult)

        res_t = persist.tile([NBD, DD], F32, tag="res")
        nc.vector.tensor_scalar(res_t[:], s_diag[:], fac[:], None, OP.mult)

        nc.sync.dma_start(out_ap.rearrange("b d j -> (b d) j"), res_t[:])


_CACHE: dict = {}


def _get_nc():
    if "nc" not in _CACHE:
        nc = bacc.Bacc(
            "TRN2", target_bir_lowering=False, debug=False, num_devices=NCORES
        )
        # host-pre-arranged: W, u and B_prior fused per tile so each tile is
        # ONE fully contiguous DMA (cols 0:1280 = W, 1280:1296 = u, 1296:1306 = bp)
        WUB = nc.dram_tensor(
            "wub_arr", [NT, 128, FW + BPC * DP + D], F32, kind="ExternalInput"
        ).ap()
        out = nc.dram_tensor("out", [BPC, D, DD], F32, kind="ExternalOutput").ap()
        with tile.TileContext(nc) as tc:
            _build_kernel(tc, out, WUB)
        nc.compile()
        _CACHE["nc"] = nc
    return _CACHE["nc"]


def _arrange(primary_caps, W, B_prior, core):
    """Host-side pre-arrangement into the exact SBUF tile layouts so every
    device DMA reads fully contiguous memory."""
    W = np.asarray(W, dtype=np.float32)
    Bp = np.asarray(B_prior, dtype=np.float32)
    pc = np.asarray(primary_caps, dtype=np.float32)
    w_arr = W.transpose(1, 0, 2, 3).reshape(NT, 128, FW)
    u_arr = (
        pc[core * BPC:(core + 1) * BPC]
        .transpose(1, 0, 2)
        .reshape(NT, 128, BPC * DP)
    )
    bp_arr = Bp[:, 0, :].T.reshape(NT, 128, D)
    return {
        "wub_arr": np.ascontiguousarray(
            np.concatenate([w_arr, u_arr, bp_arr], axis=2)
        )
    }


def _run(primary_caps, W, B_prior, trace=False, **kw):
    nc = _get_nc()
    in_maps = [
        _arrange(primary_caps, W, B_prior, c) for c in range(NCORES)
    ]
    res = run_bass_kernel_spmd(nc, in_maps, list(range(NCORES)), trace=trace, **kw)
    out = np.concatenate([res.results[c]["out"] for c in range(NCORES)], axis=0)
    return out.astype(np.float32), res


def kernel(primary_caps, W, B_prior):
    out, _ = _run(primary_caps, W, B_prior, trace=False)
    return out



# revision 3
# speedup vs baseline: 1.0166x; 1.0166x over previous
"""DigitCaps kernel for 8 Trainium2 NeuronCores (fp16 rewrite).

Math (per batch b):
    U_hat[b,d,n,j] = sum_i W[d,n,j,i] * u[b,n,i]
    A_sum[b,d,m]   = s[b,d,:] . U_hat[b,d,m,:] / sqrt(dp),  s = sum_n U_hat
    C              = softmax_d(A_sum)
    S[b,d,j]       = sum_m (B_prior[d,m] + C[b,d,m]) * U_hat[b,d,m,j]
    out            = squash(S)

Sharding: data-parallel over batch, 2 batches per core, W/B_prior replicated.
All heavy tensors are fp16 on device: halves the DMA stream (the memory
roofline) and doubles DVE throughput (2x mode needs 2-byte packed operands).

Per-core layout: n-tiles of 128 on partitions, W as [n, (d,j, i)] so the
products TT reads contiguous fp16 and the i-tree adds stay stride-1 until
the last level.
    phase 1 per tile: products P[n,(b,dj,i)] (one DVE TT with broadcast
    views, or 16 ACT copy-scale, or a Pool STT chain for whole tiles),
    i-tree on DVE, running s via fp16 PE ones-matmul (PSUM accumulate).
    phase 2 (3 chunks of 3 tiles): TA = U2*s_bc -> j-reduce -> exp(ACT)
    -> softmax_d norm -> +B_prior -> S2 matmul (fp16, PSUM accum).
    phase 3/4: diagonal extract via iota mask; squash with Halley sqrt.
"""

import math
import numpy as np

import concourse.bacc as bacc
import concourse.bass as bass
import concourse.tile as tile
from concourse import mybir
from concourse.bass_utils import run_bass_kernel_spmd

F32 = mybir.dt.float32
F16 = mybir.dt.float16
I32 = mybir.dt.int32
AX = mybir.AxisListType
OP = mybir.AluOpType
ACTF = mybir.ActivationFunctionType

B, N, DP = 16, 1152, 8
D, DD = 10, 16
NCORES = 8
BPC = B // NCORES            # 2 batches per core
NT = N // 128                # 9 n-tiles
FD = D * DD                  # 160 per-batch free size (d,j)
FW = FD * DP                 # 1280 W free size
FU = BPC * FD                # 320 U2 free size
FP = BPC * FW                # 2560 products free size
NBD = BPC * D                # 20 (b,d) pairs
EPS = 1e-7
INV_SQRT_DP = 1.0 / math.sqrt(DP)
# per-tile fp16 row: W 1280 | u16 16 | bp 10 ; plus u32 (fp32) 16 at the tail
ROW16 = FW + BPC * DP + D
ROWB = ROW16 * 2 + BPC * DP * 4   # bytes per partition per tile

# engine assignment per n-tile
ACT_TILES = (0, 2)            # products on ACT (16 copy-scale), tree on DVE
POOL_TILES = (1, 3)           # whole tile on Pool via STT chain
POOL_TREE_TILES = (4,)        # products on DVE, tree on Pool


def _build_kernel(tc: "tile.TileContext", out_ap, WUB, WU32):
    nc = tc.nc
    with (
        tc.tile_pool(name="wpool", bufs=NT) as wpool,
        tc.tile_pool(name="ppool", bufs=3) as ppool,
        tc.tile_pool(name="tapool", bufs=4) as tapool,
        tc.tile_pool(name="persist", bufs=1) as persist,
        tc.tile_pool(name="psum_s", bufs=1, space="PSUM") as psum_s,
        tc.tile_pool(name="psum_S2", bufs=1, space="PSUM") as psum_S2,
    ):
        ones_t = persist.tile([128, 128], F16, name="ones_t")
        nc.vector.memset(ones_t[:], 1.0)

        u2_all = persist.tile([128, NT * FU], F16, name="u2_all")
        s_ps = psum_s.tile([128, FU], F32, name="s_ps")
        S2_ps = psum_S2.tile([NBD, FU], F32, name="S2_ps")

        w16 = []
        u32 = []
        for nt in range(NT):
            w_t = wpool.tile([128, ROW16], F16, tag="w16")
            u_t = wpool.tile([128, BPC * DP], F32, tag="u32")
            w16.append(w_t)
            u32.append(u_t)
            nc.sync.dma_start(w_t[:], WUB[nt])
            nc.sync.dma_start(u_t[:], WU32[nt])

        with nc.allow_low_precision(reason="fp16 kernel, tol 2e-2"):
            for nt in range(NT):
                w_t = w16[nt]
                u_t = u32[nt]
                w_v = w_t[:, :FW].rearrange("p (dj i) -> p dj i", dj=FD, i=DP)
                u16 = w_t[:, FW: FW + BPC * DP]
                u2_sl = u2_all[:, nt * FU:(nt + 1) * FU]

                if nt in POOL_TILES:
                    # whole tile on Pool: TS + 7 STT fused MACs per batch
                    for b in range(BPC):
                        acc = u2_sl[:, b * FD:(b + 1) * FD]
                        nc.gpsimd.tensor_scalar(
                            acc, w_v[:, :, 0],
                            u_t[:, b * DP: b * DP + 1], None, OP.mult,
                        )
                        for i in range(1, DP):
                            nc.gpsimd.scalar_tensor_tensor(
                                acc, w_v[:, :, i],
                                u_t[:, b * DP + i: b * DP + i + 1],
                                acc, OP.mult, OP.add,
                            )
                else:
                    pp = ppool.tile([128, FP], F16, tag="pp")
                    pp_v = pp[:].rearrange(
                        "p (b dj i) -> p b dj i", b=BPC, dj=FD, i=DP
                    )
                    if nt in ACT_TILES:
                        # products on ACT: copy with per-partition scale
                        for b in range(BPC):
                            for i in range(DP):
                                nc.scalar.activation(
                                    pp_v[:, b, :, i],
                                    w_v[:, :, i],
                                    ACTF.Copy,
                                    scale=u_t[:, b * DP + i: b * DP + i + 1],
                                )
                    else:
                        # products in one DVE TT via broadcast views (2x)
                        w_bc = (
                            w_t[:, :FW]
                            .rearrange("p (dj i) -> p dj i", dj=FD, i=DP)
                            .unsqueeze(1)
                            .broadcast_to([128, BPC, FD, DP])
                        )
                        u_bc = (
                            u16.rearrange("p (b i) -> p b i", b=BPC, i=DP)
                            .unsqueeze(2)
                            .broadcast_to([128, BPC, FD, DP])
                        )
                        nc.vector.tensor_tensor(pp_v, w_bc, u_bc, OP.mult)

                    # i-tree: contiguous-halves adds (2x until last level)
                    eng = nc.gpsimd if nt in POOL_TREE_TILES else nc.vector
                    pg = pp[:].rearrange("p (g i) -> p g i", g=FU, i=DP)
                    t1 = ppool.tile([128, FU * 4], F16, tag="t1")
                    t1_v = t1[:].rearrange("p (g i) -> p g i", g=FU, i=4)
                    eng.tensor_tensor(t1_v, pg[:, :, 0:4], pg[:, :, 4:8], OP.add)
                    t2 = ppool.tile([128, FU * 2], F16, tag="t2")
                    t2_v = t2[:].rearrange("p (g i) -> p g i", g=FU, i=2)
                    eng.tensor_tensor(t2_v, t1_v[:, :, 0:2], t1_v[:, :, 2:4], OP.add)
                    eng.tensor_tensor(
                        u2_sl.rearrange("p (g i) -> p g i", g=FU, i=1),
                        t2_v[:, :, 0:1], t2_v[:, :, 1:2], OP.add,
                    )

                # running s: ones^T @ U2 accumulated in PSUM (fp16 operands)
                nc.tensor.matmul(
                    s_ps[:], ones_t[:], u2_sl,
                    start=(nt == 0), stop=(nt == NT - 1),
                )

            # ---- phase 2: 3 chunks of 3 tiles ----
            s_sb = persist.tile([128, FU], F16, name="s_sb")
            nc.vector.tensor_copy(s_sb[:], s_ps[:])

            NC = 3            # tiles per chunk
            CH = NT // NC     # 3 chunks
            e_all = persist.tile([128, NT * NBD], F16, name="e_all")
            z_all = persist.tile([128, NT * BPC], F16, name="z_all")
            zr_all = persist.tile([128, NT * BPC], F16, name="zr_all")
            cb_all = persist.tile([128, NT * NBD], F16, name="cb_all")

            for c in range(CH):
                lo, hi = c * NC, (c + 1) * NC
                ta = tapool.tile([128, NC * FU], F16, tag="ta")
                ta_v = ta[:].rearrange("p (t f) -> p t f", t=NC, f=FU)
                s_bc = s_sb[:].unsqueeze(1).broadcast_to([128, NC, FU])
                nc.vector.tensor_tensor(
                    ta_v, u2_all[:, lo * FU: hi * FU].rearrange(
                        "p (t f) -> p t f", t=NC, f=FU),
                    s_bc, OP.mult,
                )
                a_sl = e_all[:, lo * NBD: hi * NBD]
                nc.vector.tensor_reduce(
                    a_sl,
                    ta[:].rearrange("p (g j) -> p g j", g=NC * NBD, j=DD),
                    AX.X, OP.add,
                )
                # E = exp(A / sqrt(dp))
                nc.scalar.activation(a_sl, a_sl, ACTF.Exp, scale=INV_SQRT_DP)
                # z = sum_d E ; zr = 1/z
                z_sl = z_all[:, lo * BPC: hi * BPC]
                zr_sl = zr_all[:, lo * BPC: hi * BPC]
                nc.vector.tensor_reduce(
                    z_sl,
                    a_sl.rearrange("p (g d) -> p g d", g=NC * BPC, d=D),
                    AX.X, OP.add,
                )
                nc.vector.reciprocal(zr_sl, z_sl)
                # cb = E * zr + B_prior
                cb_sl = cb_all[:, lo * NBD: hi * NBD]
                cb_v = cb_sl.rearrange("p (g d) -> p g d", g=NC * BPC, d=D)
                zr_bc = (
                    zr_sl.rearrange("p (g o) -> p g o", g=NC * BPC, o=1)
                    .broadcast_to([128, NC * BPC, D])
                )
                nc.vector.tensor_tensor(
                    cb_v,
                    a_sl.rearrange("p (g d) -> p g d", g=NC * BPC, d=D),
                    zr_bc, OP.mult,
                )
                for t in range(lo, hi):
                    bp = w16[t][:, FW + BPC * DP: FW + BPC * DP + D]
                    bp_bc = bp.unsqueeze(1).broadcast_to([128, BPC, D])
                    cbt = cb_all[:, t * NBD:(t + 1) * NBD].rearrange(
                        "p (b d) -> p b d", b=BPC, d=D)
                    nc.vector.tensor_tensor(cbt, cbt, bp_bc, OP.add)
                    # S2 += cb.T @ U2 (fp16 operands, fp32 PSUM accumulate)
                    nc.tensor.matmul(
                        S2_ps[:],
                        cb_all[:, t * NBD:(t + 1) * NBD],
                        u2_all[:, t * FU:(t + 1) * FU],
                        start=(t == 0), stop=(t == NT - 1),
                    )

            # ---- phase 3: extract diagonal (b,d)=(b',d') via iota mask ----
            iota_t = persist.tile([NBD, FU], I32, name="iota_t")
            nc.gpsimd.iota(
                iota_t[:], pattern=[[1, NBD], [0, DD]], base=0,
                channel_multiplier=-1,
            )
            mask_t = persist.tile([NBD, FU], F32, name="mask_t")
            nc.vector.tensor_scalar(mask_t[:], iota_t[:], 0, None, OP.is_equal)

            sm_t = persist.tile([NBD, FU], F32, name="sm_t")
            nc.vector.tensor_tensor(sm_t[:], S2_ps[:], mask_t[:], OP.mult)
            s_diag = persist.tile([NBD, DD], F32, name="s_diag")
            nc.vector.tensor_reduce(
                s_diag[:],
                sm_t[:].rearrange("p (g j) -> p j g", g=NBD, j=DD),
                AX.X, OP.add,
            )

        # ---- phase 4: squash (fp32) ----
        ss_t = persist.tile([NBD, DD], F32, name="ss_t")
        nrm2 = persist.tile([NBD, 1], F32, name="nrm2")
        nc.vector.tensor_tensor(ss_t[:], s_diag[:], s_diag[:], OP.mult)
        nc.vector.tensor_reduce(nrm2[:], ss_t[:], AX.X, OP.add)
        # norm via one Halley iteration from the bit-hack seed (cubic:
        # 3.5e-2 seed error -> ~4e-5), all on DVE
        nrm = persist.tile([NBD, 1], F32, name="nrm")
        seed_i = persist.tile([NBD, 1], I32, name="seed_i")
        nc.vector.tensor_scalar(
            seed_i[:], nrm2[:].bitcast(I32), 1, None, OP.logical_shift_right
        )
        nc.vector.tensor_scalar(seed_i[:], seed_i[:], 0x1FBD1DF5, None, OP.add)
        seed_f = seed_i[:].bitcast(F32)
        y2 = persist.tile([NBD, 1], F32, name="y2")
        nc.vector.tensor_tensor(y2[:], seed_f, seed_f, OP.mult)
        hnum = persist.tile([NBD, 1], F32, name="hnum")
        nc.vector.scalar_tensor_tensor(hnum[:], nrm2[:], 3.0, y2[:], OP.mult, OP.add)
        hden = persist.tile([NBD, 1], F32, name="hden")
        nc.vector.scalar_tensor_tensor(hden[:], y2[:], 3.0, nrm2[:], OP.mult, OP.add)
        nwr = persist.tile([NBD, 1], F32, name="nwr")
        nc.vector.reciprocal(nwr[:], hden[:])
        nwt = persist.tile([NBD, 1], F32, name="nwt")
        nc.vector.tensor_tensor(nwt[:], hnum[:], nwr[:], OP.mult)
        nc.vector.tensor_tensor(nrm[:], seed_f, nwt[:], OP.mult)
        # coef = 1 - 1/(e^r + eps) ~= 1 - e^-r  (abs diff <= eps*e^-2r <= 1e-7)
        en = persist.tile([NBD, 1], F32, name="en")
        nc.scalar.activation(en[:], nrm[:], ACTF.Exp, scale=-1.0)
        coef = persist.tile([NBD, 1], F32, name="coef")
        nc.vector.tensor_scalar(coef[:], en[:], -1.0, 1.0, OP.mult, OP.add)
        r2 = persist.tile([NBD, 1], F32, name="r2")
        nc.vector.reciprocal(r2[:], nrm[:])
        fac = persist.tile([NBD, 1], F32, name="fac")
        nc.vector.tensor_tensor(fac[:], coef[:], r2[:], OP.mult)

        res_t = persist.tile([NBD, DD], F32, name="res_t")
        nc.vector.tensor_scalar(res_t[:], s_diag[:], fac[:], None, OP.mult)

        nc.sync.dma_start(out_ap.rearrange("b d j -> (b d) j"), res_t[:])


_CACHE: dict = {}


def _get_nc():
    if "nc" not in _CACHE:
        nc = bacc.Bacc(
            "TRN2", target_bir_lowering=False, debug=False, num_devices=NCORES
        )
        # host-pre-arranged per tile: fp16 row = W (dj,i-major) | u fp16 | bp,
        # plus a small fp32 copy of u for ACT scales / Pool STT scalars.
        WUB = nc.dram_tensor(
            "wub_arr", [NT, 128, ROW16], F16, kind="ExternalInput"
        ).ap()
        WU32 = nc.dram_tensor(
            "u32_arr", [NT, 128, BPC * DP], F32, kind="ExternalInput"
        ).ap()
        out = nc.dram_tensor("out", [BPC, D, DD], F32, kind="ExternalOutput").ap()
        with tile.TileContext(nc) as tc:
            _build_kernel(tc, out, WUB, WU32)
        nc.compile()
        _CACHE["nc"] = nc
    return _CACHE["nc"]


def _arrange(primary_caps, W, B_prior, core):
    """Host-side pre-arrangement into the exact SBUF tile layouts so every
    device DMA reads fully contiguous memory."""
    W = np.asarray(W, dtype=np.float32)
    Bp = np.asarray(B_prior, dtype=np.float32)
    pc = np.asarray(primary_caps, dtype=np.float32)
    # W[d,n,j,i] -> [nt, n128, (d,j,i)] fp16
    w_arr = (
        W.transpose(1, 0, 2, 3).reshape(NT, 128, FW).astype(np.float16)
    )
    u_blk = (
        pc[core * BPC:(core + 1) * BPC]
        .transpose(1, 0, 2)
        .reshape(NT, 128, BPC * DP)
    )
    bp_arr = Bp[:, 0, :].T.reshape(NT, 128, D).astype(np.float16)
    return {
        "wub_arr": np.ascontiguousarray(
            np.concatenate([w_arr, u_blk.astype(np.float16), bp_arr], axis=2)
        ),
        "u32_arr": np.ascontiguousarray(u_blk),
    }


def _run(primary_caps, W, B_prior, trace=False, **kw):
    nc = _get_nc()
    in_maps = [
        _arrange(primary_caps, W, B_prior, c) for c in range(NCORES)
    ]
    res = run_bass_kernel_spmd(nc, in_maps, list(range(NCORES)), trace=trace, **kw)
    out = np.concatenate([res.results[c]["out"] for c in range(NCORES)], axis=0)
    return out.astype(np.float32), res


def kernel(primary_caps, W, B_prior):
    out, _ = _run(primary_caps, W, B_prior, trace=False)
    return out


# revision 5
# speedup vs baseline: 1.3806x; 1.3581x over previous
"""DigitCaps kernel for 8 Trainium2 NeuronCores (fp16, (n,i)-layout).

Math (per batch b):
    U_hat[b,d,n,j] = sum_i W[d,n,j,i] * u[b,n,i]
    A_sum[b,d,m]   = s[b,d,:] . U_hat[b,d,m,:] / sqrt(dp),  s = sum_n U_hat
    C              = softmax_d(A_sum)
    S[b,d,j]       = sum_m (B_prior[d,m] + C[b,d,m]) * U_hat[b,d,m,j]
    out            = squash(S)

Sharding: data-parallel over batch, 2 batches per core, W/B_prior replicated.

Key layout trick: W tiles are host-arranged with partitions = (n16, i8) so
the per-(chunk,batch) vote products are single tensor_scalar ops (fp16 4x
mode on DVE; ACT copy-scale / Pool TS take a share), and the i-contraction
runs on the otherwise-idle PE: one block-delta matmul per chunk accumulates
U2 back into n-major partitions in PSUM. No adder trees on the vector
engines at all. The B_prior part of S folds into phase-1 PE matmuls
(S2 += bp.T @ U2) so the tail only handles the softmax part.
"""

import math
import numpy as np

import concourse.bacc as bacc
import concourse.bass as bass
import concourse.tile as tile
from concourse import mybir
from concourse.bass_utils import run_bass_kernel_spmd

F32 = mybir.dt.float32
F16 = mybir.dt.float16
I32 = mybir.dt.int32
AX = mybir.AxisListType
OP = mybir.AluOpType
ACTF = mybir.ActivationFunctionType

B, N, DP = 16, 1152, 8
D, DD = 10, 16
NCORES = 8
BPC = B // NCORES            # 2 batches per core
NT = N // 128                # 9 n-tiles
NG = 128 // 16               # 8 chunks of 16 n per tile
FD = D * DD                  # 160 per-batch free size (d,j)
FW = FD * DP                 # 1280 W free size per partition
FU = BPC * FD                # 320 U2 free size
NBD = BPC * D                # 20 (b,d) pairs
INV_SQRT_DP = 1.0 / math.sqrt(DP)
ROW16 = NG * FD + NBD        # W_ni 1280 | bp_bd 20

# per-(tile,chunk) product engine: 'D' (DVE TS), 'A' (ACT copy-scale),
# 'P' (Pool TS).  ACT ~16 chunks, Pool ~19, DVE ~37.
_SEQ = ("A", "P", "D", "D", "P", "D", "D", "D")
CHUNK_ENG = {}
for _t in range(NT):
    for _g in range(NG):
        e = _SEQ[(_g + _t * 3) % NG]
        # thin ACT/Pool a bit on later tiles to land near 16/19/37
        if e == "A" and _t in (3, 7):
            e = "D"
        CHUNK_ENG[(_t, _g)] = e
COPY_ENG = ("D", "A", "D", "A", "D", "A", "D", "A", "D")


def _build_kernel(tc: "tile.TileContext", out_ap, WMAIN, UAUX):
    nc = tc.nc
    with (
        tc.tile_pool(name="wpool", bufs=NT) as wpool,
        tc.tile_pool(name="ppool", bufs=3) as ppool,
        tc.tile_pool(name="tapool", bufs=4) as tapool,
        tc.tile_pool(name="persist", bufs=1) as persist,
        tc.tile_pool(name="psum_u2", bufs=3, space="PSUM") as psum_u2,
        tc.tile_pool(name="psum_s", bufs=1, space="PSUM") as psum_s,
        tc.tile_pool(name="psum_S2", bufs=1, space="PSUM") as psum_S2,
    ):
        # ---- t0 pre-work (overlaps the first DMAs) ----
        # load the exp_and_others ACT table once: covers Copy and Exp
        warm_t = persist.tile([1, 1], F32, name="warm_t")
        nc.vector.memset(warm_t[:], 0.0)
        nc.scalar.activation(warm_t[:], warm_t[:], ACTF.Exp)

        ones_t = persist.tile([128, 128], F16, name="ones_t")
        nc.vector.memset(ones_t[:], 1.0)

        # delta_g[c, p] = 1 iff p == g*16 + c//8   (c = n*8 + i)
        c_i = persist.tile([128, 1], I32, name="c_i")
        nc.gpsimd.iota(c_i[:], pattern=[[0, 1]], base=0, channel_multiplier=1)
        c8 = persist.tile([128, 1], I32, name="c8")
        nc.vector.tensor_scalar(c8[:], c_i[:], 3, None, OP.logical_shift_right)
        pcol = persist.tile([128, 128], I32, name="pcol")
        nc.gpsimd.iota(pcol[:], pattern=[[1, 128]], base=0, channel_multiplier=0)
        diff = persist.tile([128, 128], I32, name="diff")
        nc.vector.tensor_tensor(
            diff[:], pcol[:], c8[:].broadcast_to([128, 128]), OP.subtract
        )
        deltas = persist.tile([128, NG * 128], F16, name="deltas")
        for g in range(NG):
            nc.vector.tensor_scalar(
                deltas[:, g * 128:(g + 1) * 128], diff[:], g * 16, None,
                OP.is_equal,
            )

        # diag mask for phase 3 (iota trick)
        iota_t = persist.tile([NBD, FU], I32, name="iota_t")
        nc.gpsimd.iota(
            iota_t[:], pattern=[[1, NBD], [0, DD]], base=0,
            channel_multiplier=-1,
        )
        mask_t = persist.tile([NBD, FU], F32, name="mask_t")
        nc.vector.tensor_scalar(mask_t[:], iota_t[:], 0, None, OP.is_equal)

        u2_all = persist.tile([128, NT * FU], F16, name="u2_all")
        s_ps = psum_s.tile([128, FU], F32, name="s_ps")
        S2_ps = psum_S2.tile([NBD, FU], F32, name="S2_ps")

        # ---- DMAs ----
        w16 = []
        u32 = []
        for nt in range(NT):
            w_t = wpool.tile([128, ROW16], F16, tag="w16")
            u_t = wpool.tile([128, NG * BPC], F32, tag="u32")
            w16.append(w_t)
            u32.append(u_t)
            nc.sync.dma_start(w_t[:], WMAIN[nt])
            nc.sync.dma_start(u_t[:], UAUX[nt])

        with nc.allow_low_precision(reason="fp16 kernel, tol 2e-2"):
            # ---- phase 1: products (TS) + PE delta-matmul i-reduction ----
            pe_q = []      # deferred PE emission: ("d", nt) / ("s", nt) / ("b", nt)
            u2_ps_t = {}
            pp_t = {}
            for nt in range(NT):
                w_t = w16[nt]
                u_t = u32[nt]
                pp = ppool.tile([128, NG * FU], F16, tag="pp")
                pp_t[nt] = pp
                for g in range(NG):
                    eng = CHUNK_ENG[(nt, g)]
                    for b in range(BPC):
                        w_sl = w_t[:, g * FD:(g + 1) * FD]
                        o_sl = pp[:, g * FU + b * FD: g * FU + (b + 1) * FD]
                        sc = u_t[:, g * BPC + b: g * BPC + b + 1]
                        if eng == "A":
                            nc.scalar.activation(
                                o_sl, w_sl, ACTF.Copy, scale=sc)
                        elif eng == "P":
                            nc.gpsimd.tensor_scalar(
                                o_sl, w_sl, sc, None, OP.mult)
                        else:
                            nc.vector.tensor_scalar(
                                o_sl, w_sl, sc, None, OP.mult)
                u2_ps = psum_u2.tile([128, FU], F32, tag="u2ps")
                u2_ps_t[nt] = u2_ps
                pe_q.append(("d", nt))
                if nt >= 1:
                    pe_q.append(("s", nt - 1))
                    pe_q.append(("b", nt - 1))
            pe_q += [("s", NT - 1), ("b", NT - 1)]

            # emit PE + copies interleaved in dependency-friendly order
            s_first = True
            b_first = True
            copied = {}
            for kind, nt in pe_q:
                if kind == "d":
                    for g in range(NG):
                        nc.tensor.matmul(
                            u2_ps_t[nt][:],
                            deltas[:, g * 128:(g + 1) * 128],
                            pp_t[nt][:, g * FU:(g + 1) * FU],
                            start=(g == 0), stop=(g == NG - 1),
                        )
                    # copy PSUM -> SBUF fp16 (DVE/ACT alternate)
                    u2_sl = u2_all[:, nt * FU:(nt + 1) * FU]
                    if COPY_ENG[nt] == "A":
                        nc.scalar.copy(u2_sl, u2_ps_t[nt][:])
                    else:
                        nc.vector.tensor_copy(u2_sl, u2_ps_t[nt][:])
                    copied[nt] = True
                elif kind == "s":
                    nc.tensor.matmul(
                        s_ps[:], ones_t[:],
                        u2_all[:, nt * FU:(nt + 1) * FU],
                        start=s_first, stop=(nt == NT - 1),
                    )
                    s_first = False
                else:  # S2 += bp.T @ U2  (B_prior part of S, done in phase 1)
                    nc.tensor.matmul(
                        S2_ps[:],
                        w16[nt][:, NG * FD: NG * FD + NBD],
                        u2_all[:, nt * FU:(nt + 1) * FU],
                        start=b_first, stop=False,
                    )
                    b_first = False

            # ---- phase 2: 3 chunks of 3 tiles ----
            s_sb = persist.tile([128, FU], F16, name="s_sb")
            nc.vector.tensor_copy(s_sb[:], s_ps[:])

            NC = 3
            e_all = persist.tile([128, NT * NBD], F16, name="e_all")
            z_all = persist.tile([128, NT * BPC], F16, name="z_all")
            zr_all = persist.tile([128, NT * BPC], F16, name="zr_all")
            cb_all = persist.tile([128, NT * NBD], F16, name="cb_all")

            for c in range(NT // NC):
                lo, hi = c * NC, (c + 1) * NC
                ta = tapool.tile([128, NC * FU], F16, tag="ta")
                ta_v = ta[:].rearrange("p (t f) -> p t f", t=NC, f=FU)
                s_bc = s_sb[:].unsqueeze(1).broadcast_to([128, NC, FU])
                nc.vector.tensor_tensor(
                    ta_v, u2_all[:, lo * FU: hi * FU].rearrange(
                        "p (t f) -> p t f", t=NC, f=FU),
                    s_bc, OP.mult,
                )
                # j-reduction as a 2x-mode contiguous-halves tree
                G = NC * NBD  # 60 groups of 16 j
                tg = ta[:].rearrange("p (g j) -> p g j", g=G, j=DD)
                l1 = tapool.tile([128, G * 8], F16, tag="l1")
                l1v = l1[:].rearrange("p (g j) -> p g j", g=G, j=8)
                nc.vector.tensor_tensor(l1v, tg[:, :, 0:8], tg[:, :, 8:16], OP.add)
                l2 = tapool.tile([128, G * 4], F16, tag="l2")
                l2v = l2[:].rearrange("p (g j) -> p g j", g=G, j=4)
                nc.vector.tensor_tensor(l2v, l1v[:, :, 0:4], l1v[:, :, 4:8], OP.add)
                l3 = tapool.tile([128, G * 2], F16, tag="l3")
                l3v = l3[:].rearrange("p (g j) -> p g j", g=G, j=2)
                nc.vector.tensor_tensor(l3v, l2v[:, :, 0:2], l2v[:, :, 2:4], OP.add)
                a_sl = e_all[:, lo * NBD: hi * NBD]
                nc.vector.tensor_tensor(
                    a_sl.rearrange("p (g j) -> p g j", g=G, j=1),
                    l3v[:, :, 0:1], l3v[:, :, 1:2], OP.add,
                )
                # E = exp(A / sqrt(dp))
                nc.scalar.activation(a_sl, a_sl, ACTF.Exp, scale=INV_SQRT_DP)
                # z = sum_d E ; zr = 1/z ; cb = E * zr
                z_sl = z_all[:, lo * BPC: hi * BPC]
                zr_sl = zr_all[:, lo * BPC: hi * BPC]
                nc.vector.tensor_reduce(
                    z_sl,
                    a_sl.rearrange("p (g d) -> p g d", g=NC * BPC, d=D),
                    AX.X, OP.add,
                )
                nc.vector.reciprocal(zr_sl, z_sl)
                cb_sl = cb_all[:, lo * NBD: hi * NBD]
                zr_bc = (
                    zr_sl.rearrange("p (g o) -> p g o", g=NC * BPC, o=1)
                    .broadcast_to([128, NC * BPC, D])
                )
                nc.vector.tensor_tensor(
                    cb_sl.rearrange("p (g d) -> p g d", g=NC * BPC, d=D),
                    a_sl.rearrange("p (g d) -> p g d", g=NC * BPC, d=D),
                    zr_bc, OP.mult,
                )
                for t in range(lo, hi):
                    nc.tensor.matmul(
                        S2_ps[:],
                        cb_all[:, t * NBD:(t + 1) * NBD],
                        u2_all[:, t * FU:(t + 1) * FU],
                        start=False, stop=(t == NT - 1),
                    )

            # ---- phase 3: extract diagonal (b,d)=(b',d') ----
            sm_t = persist.tile([NBD, FU], F32, name="sm_t")
            nc.vector.tensor_tensor(sm_t[:], S2_ps[:], mask_t[:], OP.mult)
            s_diag = persist.tile([NBD, DD], F32, name="s_diag")
            nc.vector.tensor_reduce(
                s_diag[:],
                sm_t[:].rearrange("p (g j) -> p j g", g=NBD, j=DD),
                AX.X, OP.add,
            )

        # ---- phase 4: squash (fp32) ----
        ss_t = persist.tile([NBD, DD], F32, name="ss_t")
        nrm2 = persist.tile([NBD, 1], F32, name="nrm2")
        nc.vector.tensor_tensor(ss_t[:], s_diag[:], s_diag[:], OP.mult)
        nc.vector.tensor_reduce(nrm2[:], ss_t[:], AX.X, OP.add)
        nrm = persist.tile([NBD, 1], F32, name="nrm")
        seed_i = persist.tile([NBD, 1], I32, name="seed_i")
        nc.vector.tensor_scalar(
            seed_i[:], nrm2[:].bitcast(I32), 1, None, OP.logical_shift_right
        )
        nc.vector.tensor_scalar(seed_i[:], seed_i[:], 0x1FBD1DF5, None, OP.add)
        seed_f = seed_i[:].bitcast(F32)
        y2 = persist.tile([NBD, 1], F32, name="y2")
        nc.vector.tensor_tensor(y2[:], seed_f, seed_f, OP.mult)
        hnum = persist.tile([NBD, 1], F32, name="hnum")
        nc.vector.scalar_tensor_tensor(hnum[:], nrm2[:], 3.0, y2[:], OP.mult, OP.add)
        hden = persist.tile([NBD, 1], F32, name="hden")
        nc.vector.scalar_tensor_tensor(hden[:], y2[:], 3.0, nrm2[:], OP.mult, OP.add)
        nwr = persist.tile([NBD, 1], F32, name="nwr")
        nc.vector.reciprocal(nwr[:], hden[:])
        nwt = persist.tile([NBD, 1], F32, name="nwt")
        nc.vector.tensor_tensor(nwt[:], hnum[:], nwr[:], OP.mult)
        nc.vector.tensor_tensor(nrm[:], seed_f, nwt[:], OP.mult)
        en = persist.tile([NBD, 1], F32, name="en")
        nc.scalar.activation(en[:], nrm[:], ACTF.Exp, scale=-1.0)
        coef = persist.tile([NBD, 1], F32, name="coef")
        nc.vector.tensor_scalar(coef[:], en[:], -1.0, 1.0, OP.mult, OP.add)
        r2 = persist.tile([NBD, 1], F32, name="r2")
        nc.vector.reciprocal(r2[:], nrm[:])
        fac = persist.tile([NBD, 1], F32, name="fac")
        nc.vector.tensor_tensor(fac[:], coef[:], r2[:], OP.mult)

        res_t = persist.tile([NBD, DD], F32, name="res_t")
        nc.vector.tensor_scalar(res_t[:], s_diag[:], fac[:], None, OP.mult)

        nc.sync.dma_start(out_ap.rearrange("b d j -> (b d) j"), res_t[:])


_CACHE: dict = {}


def _get_nc():
    if "nc" not in _CACHE:
        nc = bacc.Bacc(
            "TRN2", target_bir_lowering=False, debug=False, num_devices=NCORES
        )
        WMAIN = nc.dram_tensor(
            "wmain", [NT, 128, ROW16], F16, kind="ExternalInput"
        ).ap()
        UAUX = nc.dram_tensor(
            "uaux", [NT, 128, NG * BPC], F32, kind="ExternalInput"
        ).ap()
        out = nc.dram_tensor("out", [BPC, D, DD], F32, kind="ExternalOutput").ap()
        with tile.TileContext(nc) as tc:
            _build_kernel(tc, out, WMAIN, UAUX)
        nc.compile()
        _CACHE["nc"] = nc
    return _CACHE["nc"]


def _arrange(primary_caps, W, B_prior, core):
    """Host-side pre-arrangement into the exact SBUF tile layouts so every
    device DMA reads fully contiguous memory."""
    W = np.asarray(W, dtype=np.float32)
    Bp = np.asarray(B_prior, dtype=np.float32)
    pc = np.asarray(primary_caps, dtype=np.float32)[core * BPC:(core + 1) * BPC]
    # W[d,n,j,i] with n = nt*128 + g*16 + nn -> [nt, (nn,i), (g,d,j)]
    w_ni = (
        W.transpose(1, 3, 0, 2)               # [N, i, d, j]
        .reshape(NT, NG, 16, DP, D, DD)       # [nt, g, nn, i, d, j]
        .transpose(0, 2, 3, 1, 4, 5)          # [nt, nn, i, g, d, j]
        .reshape(NT, 128, NG * FD)
        .astype(np.float16)
    )
    # bp_bd[nt][p=n, (b,d)] = Bp[d, nt*128+p]
    bp = Bp[:, 0, :].T.reshape(NT, 128, D).astype(np.float16)   # [nt, n, d]
    bp_bd = np.broadcast_to(bp[:, :, None, :], (NT, 128, BPC, D)).reshape(
        NT, 128, NBD)
    # u32[nt][p=(nn,i), (g,b)] = u[b, nt*128+g*16+nn, i]
    u_ni = (
        pc.reshape(BPC, NT, NG, 16, DP)       # [b, nt, g, nn, i]
        .transpose(1, 3, 4, 2, 0)             # [nt, nn, i, g, b]
        .reshape(NT, 128, NG * BPC)
    )
    return {
        "wmain": np.ascontiguousarray(
            np.concatenate([w_ni, bp_bd], axis=2)),
        "uaux": np.ascontiguousarray(u_ni.astype(np.float32)),
    }


def _run(primary_caps, W, B_prior, trace=False, **kw):
    nc = _get_nc()
    in_maps = [
        _arrange(primary_caps, W, B_prior, c) for c in range(NCORES)
    ]
    res = run_bass_kernel_spmd(nc, in_maps, list(range(NCORES)), trace=trace, **kw)
    out = np.concatenate([res.results[c]["out"] for c in range(NCORES)], axis=0)
    return out.astype(np.float32), res


def kernel(primary_caps, W, B_prior):
    out, _ = _run(primary_caps, W, B_prior, trace=False)
    return out
